# revision 51
# baseline (speedup 1.0000x reference)
"""Trainium2 Bass kernel for nn_Attention_66640712565145.

Attention with external K and full [h,n,n] bias:
  qv = x @ w_qv ; q,v = split(qv)
  dots = (q @ ext_k^T + ext_bias) * scale ; attn = softmax(dots)
  out = (attn @ v) @ w_out + b_out

Sharding: data-parallel over batch across 8 cores (2 batches/core), no
collectives.

Design notes (cost-model-driven; 434us -> 303us vs the v1 baseline):
- The bias add is removed from the PE entirely: the host ships
  exp(bias*scale) in bf16 and the unnormalized attention weights are
  formed as exp(S) * ebias with the multiply on DVE (bf16 2x mode).
  This removes one full S-sized pass through the PE (~82us/core).
- All matmul operands are bf16 (same PE cost as fp32r at >=256 free
  columns, half the DMA/SBUF), q^T stays resident in SBUF (no DRAM
  roundtrip), and PSUM->SBUF evictions are split between Act and DVE.
- One PSUM pool layout serves all phases: ps x3 [128,1024] + po
  [65,1024] = 8 banks. h2 is processed sequentially so only one PV
  accumulator is live, freeing a third ps slot that lets QK run ahead
  of the Act-paced exp pipeline and lets projection work interleave.
- PSUM slots are granted in *emission* order, so projection psum groups
  are emitted as generators yielding every 3 matmuls and pumped from
  inside the attention mt loop ("filler"), interleaving the b=1 input
  projections and b=0 output projections into attention-phase PE gaps.
- Row sums ride along the PV matmul via a ones column in the V tile; po
  is released early by unfused drain ops (reciprocal + bf16 copy);
  normalization (broadcast matmul + in-place multiply on a_sb) runs off
  the critical path.
- ebias streams per block as [128,1024] tiles on the SP/HWDGE queue;
  each block's normalization chain is emitted one block late so its
  PSUM slot request cannot throttle the next block's QK pipeline.
- Engine busy per core (cost model): PE 265us, Act 226us, DVE 203us,
  DMA 180us; wall 303us (PE ~90% occupied over its span).
"""
import numpy as np
import ml_dtypes

HEADS = 12
DIM_HEAD = 64
SCALE = DIM_HEAD ** -0.5
B, N, DIM = 16, 1024, 768
CORES = 8
BPC = B // CORES          # batches per core
T = BPC * N               # tokens per core
NT = N // 128             # 128-row tiles per sequence
VW = DIM_HEAD + 1         # v block width incl. ones column
ET = DIM // 128           # embedding chunks (6)
HP = HEADS // 2           # head pairs (6)

_program = None


def _build_program():
    import concourse.bacc as bacc
    import concourse.mybir as mybir
    from concourse.tile import TileContext

    dt = mybir.dt
    f32, f32r, bf16 = dt.float32, dt.float32r, dt.bfloat16
    EXP = mybir.ActivationFunctionType.Exp
    CPY = mybir.ActivationFunctionType.Copy

    nc = bacc.Bacc(None, target_bir_lowering=False)

    a_xT = nc.declare_dram_parameter("xT", [BPC, DIM, N], bf16, isOutput=False)
    a_wq = nc.declare_dram_parameter("wq", [DIM, DIM], bf16, isOutput=False)
    a_wv = nc.declare_dram_parameter("wv", [DIM, DIM], bf16, isOutput=False)
    a_kT = nc.declare_dram_parameter("kT", [HP, 128, N], bf16, isOutput=False)
    a_eb = nc.declare_dram_parameter("eb", [HEADS, N, N], bf16, isOutput=False)
    a_wo = nc.declare_dram_parameter("wo", [DIM, DIM], bf16, isOutput=False)
    a_out = nc.declare_dram_parameter("out", [T, DIM], f32, isOutput=True)

    with TileContext(nc) as tc:
        with tc.tile_pool(name="persist", bufs=1) as pers, \
             tc.tile_pool(name="wq_p", bufs=1) as wqpool, \
             tc.tile_pool(name="x_p", bufs=2) as xpool, \
             tc.tile_pool(name="eb_p", bufs=22) as ebpool, \
             tc.tile_pool(name="pt_p", bufs=8) as ptpool, \
             tc.tile_pool(name="qt_p", bufs=6) as qtpool, \
             tc.tile_pool(name="so_p", bufs=3) as sopool, \
             tc.tile_pool(name="psA", bufs=3, space="PSUM") as psA, \
             tc.tile_pool(name="psO", bufs=1, space="PSUM") as psO:

            # ---------- persistent SBUF ----------
            v_sb = pers.tile([128, BPC * NT * HEADS * VW], bf16, name="v_sb")
            with nc.allow_low_precision("ones col"):
                nc.vector.memset(
                    v_sb.rearrange("p (g w) -> p g w", w=VW)[:, :, DIM_HEAD:VW]
                        .rearrange("p g w -> p (g w)"), 1.0)
            a_sb = pers.tile([128, ET * T], bf16, name="a_sb")
            kT_sb = pers.tile([128, HP * N], bf16, name="kT_sb")
            l_sb = pers.tile([128, N], f32r, name="l_sb")
            nc.vector.memset(l_sb.bitcast(f32), 1.0)
            ind_f = pers.tile([128, 128], f32, name="ind_f")
            ind_r = []
            for bb in range(BPC):
                nc.vector.memset(ind_f, 0.0)
                nc.vector.memset(ind_f[(bb * 2) * 32:(bb * 2) * 32 + 1, 0:64], 1.0)
                nc.vector.memset(
                    ind_f[(bb * 2 + 1) * 32:(bb * 2 + 1) * 32 + 1, 64:128], 1.0)
                t5 = pers.tile([128, 128], f32r, name=f"ind_r{bb}")
                with nc.allow_low_precision("exact const"):
                    nc.vector.tensor_copy(t5, ind_f)
                ind_r.append(t5)

            # ---------- input DMAs (order matters: x(b0) + wq first) ----------
            xT_t = {}
            wq_t, wv_t, wo_t = [], [], []

            def dma_x(b, ct):
                t3 = xpool.tile([128, N], bf16, name=f"xT{ct}")
                nc.sync.dma_start(out=t3, in_=a_xT[b, ct * 128:(ct + 1) * 128, :])
                xT_t[(b, ct)] = t3

            dma_x(0, 0)
            for ct in range(ET):
                t1 = wqpool.tile([128, DIM], bf16, name=f"wq{ct}")
                nc.sync.dma_start(out=t1, in_=a_wq[ct * 128:(ct + 1) * 128, :])
                wq_t.append(t1)
            for ct in range(1, ET):
                dma_x(0, ct)
            nc.sync.dma_start(out=kT_sb[:, 0:N], in_=a_kT[0, :, :])
            for ct in range(ET):
                t2 = wqpool.tile([128, DIM], bf16, name=f"wv{ct}")
                nc.sync.dma_start(out=t2, in_=a_wv[ct * 128:(ct + 1) * 128, :])
                wv_t.append(t2)
            for hp in range(1, HP):
                nc.sync.dma_start(out=kT_sb[:, hp * N:(hp + 1) * N],
                                  in_=a_kT[hp, :, :])
            for ct in range(ET):
                dma_x(1, ct)
            for et in range(ET):
                t4 = wqpool.tile([128, DIM], bf16, name=f"wo{et}")
                nc.sync.dma_start(out=t4, in_=a_wo[et * 128:(et + 1) * 128, :])
                wo_t.append(t4)

            # ---------- emission helpers ----------
            # Projection groups are generators yielding every few matmuls so
            # their PE work can be sprinkled between attention tiles (PE pulls
            # ready work in priority=emission order, so a contiguous 12-matmul
            # group would stall the next QK by ~2.6us and starve Act).
            qt = {}

            def gen_q(b, et, prio=-150):
                pq = psA.tile([128, N], f32, name="ps")
                qt[(b, et)] = qtpool.tile([128, N], bf16, name="qt")
                k = 0
                for ct in range(ET):
                    for nch in range(2):
                        nsl = slice(nch * 512, (nch + 1) * 512)
                        with tc.high_priority(offset=prio):
                            nc.tensor.matmul(
                                pq[:, nsl],
                                wq_t[ct][:, et * 128:(et + 1) * 128],
                                xT_t[(b, ct)][:, nsl],
                                start=(ct == 0), stop=(ct == ET - 1))
                        k += 1
                        if k % 3 == 0:
                            yield
                with tc.high_priority(offset=prio), \
                     nc.allow_low_precision("bf16 q"):
                    nc.scalar.activation(qt[(b, et)], pq, CPY)

            def gen_v(b, tt, prio=-150):
                pv = psA.tile([128, N], f32, name="ps")
                k = 0
                for ct in range(ET):
                    for osl in (slice(0, 512), slice(512, DIM)):
                        with tc.high_priority(offset=prio):
                            nc.tensor.matmul(
                                pv[:, osl],
                                xT_t[(b, ct)][:, tt * 128:(tt + 1) * 128],
                                wv_t[ct][:, osl],
                                start=(ct == 0), stop=(ct == ET - 1))
                        k += 1
                        if k % 3 == 0:
                            yield
                base = (b * NT + tt) * HEADS * VW
                dst3 = v_sb[:, base:base + HEADS * VW] \
                    .rearrange("p (h w) -> p h w", w=VW)
                with nc.allow_low_precision("bf16 v"):
                    nc.vector.tensor_copy(
                        dst3[:, :, 0:DIM_HEAD],
                        pv[:, 0:DIM].rearrange("p (h w) -> p h w", w=DIM_HEAD))

            def gen_o(tt, prio=-150):
                pp = psA.tile([128, N], f32, name="ps")
                k = 0
                for et in range(ET):
                    for osl in (slice(0, 512), slice(512, DIM)):
                        with tc.high_priority(offset=prio):
                            nc.tensor.matmul(
                                pp[:, osl],
                                a_sb[:, et * T + tt * 128:et * T + (tt + 1) * 128],
                                wo_t[et][:, osl],
                                start=(et == 0), stop=(et == ET - 1))
                        k += 1
                        if k % 3 == 0:
                            yield
                so = sopool.tile([128, DIM], f32, name="so")
                nc.scalar.activation(so, pp[:, 0:DIM], CPY)
                nc.sync.dma_start(out=a_out[tt * 128:(tt + 1) * 128, :], in_=so)

            _DONE = object()

            def pump(filler):
                while filler:
                    if next(filler[0], _DONE) is _DONE:
                        filler.popleft()
                    else:
                        return

            def run_all(gen):
                for _ in gen:
                    pass

            def emit_eb(hp):
                tiles = {}
                for h2 in range(2):
                    h = 2 * hp + h2
                    for mt in range(NT):
                        te = ebpool.tile([128, N], bf16, name="ebt")
                        nc.sync.dma_start(
                            out=te,
                            in_=a_eb[h, mt * 128:(mt + 1) * 128, :])
                        tiles[(h2, mt)] = te
                return tiles

            def emit_attn(hp, b, ebt, filler, pending_norm):
                tq = qt[(b, hp)]
                for h2 in range(2):
                    h = 2 * hp + h2
                    r0 = h2 * 64
                    po = psO.tile([VW, N], f32, name="po")
                    for mt in range(NT):
                        pump(filler)
                        if h2 == 0 and mt == 5 and pending_norm:
                            pending_norm.popleft()()
                        pss = psA.tile([128, N], f32, name="ps")
                        for nch in range(2):
                            nsl = slice(nch * 512, (nch + 1) * 512)
                            nc.tensor.matmul(
                                pss[:, nsl],
                                kT_sb[r0:r0 + 64,
                                      hp * N + mt * 128:hp * N + (mt + 1) * 128],
                                tq[r0:r0 + 64, nsl],
                                start=True, stop=True,
                                tile_position=(r0, 0))
                        pt = ptpool.tile([128, N], bf16, name="pt")
                        with nc.allow_low_precision("bf16 exp"):
                            nc.scalar.activation(pt, pss, EXP)
                        ebs = ebt[(h2, mt)]
                        with nc.allow_low_precision("attn*ebias"):
                            nc.vector.tensor_mul(pt, pt, ebs)
                        vbase = (b * NT + mt) * HEADS * VW + h * VW
                        for nch in range(2):
                            nsl = slice(nch * 512, (nch + 1) * 512)
                            nc.tensor.matmul(
                                po[:, nsl],
                                v_sb[:, vbase:vbase + VW],
                                pt[:, nsl],
                                start=(mt == 0), stop=(mt == NT - 1))
                    # drain: reciprocal of the ones-row + unnormalized copy
                    # (the two last readers of po — releases it for h2+1).
                    # High priority so they jump ahead of queued DVE multiplies.
                    with tc.high_priority(offset=200):
                        with nc.allow_low_precision("softmax recip"):
                            nc.vector.reciprocal(
                                l_sb[(b * 2 + h2) * 32:(b * 2 + h2) * 32 + 1, :],
                                po[DIM_HEAD:VW, :])
                        with nc.allow_low_precision("bf16 attn out"):
                            nc.vector.tensor_copy(
                                a_sb[h2 * 64:(h2 + 1) * 64,
                                     hp * T + b * N:hp * T + (b + 1) * N],
                                po[0:DIM_HEAD, :])
                def norm(hp=hp, b=b):
                    pb = psA.tile([128, N], f32, name="ps")
                    for nch in range(2):
                        nsl = slice(nch * 512, (nch + 1) * 512)
                        nc.tensor.matmul(
                            pb[:, nsl], ind_r[b], l_sb[:, nsl],
                            start=True, stop=True)
                    asl = a_sb[:, hp * T + b * N:hp * T + (b + 1) * N]
                    with nc.allow_low_precision("normalize"):
                        nc.vector.tensor_mul(asl, asl, pb)
                return norm

            # ---------- phase emission ----------
            # Block order: all of b=0's head-pairs, then all of b=1's.
            # b=1 projections and b=0 output-projections run as woven
            # "filler" psum-groups popped inside the attention mt loops
            # (PSUM slots are granted in emission order, so this is the
            # only way projection work overlaps the Act-paced pipeline).
            from collections import deque
            filler = deque([
                gen_q(0, 1), gen_v(0, 2), gen_v(0, 3),
                gen_q(0, 2), gen_v(0, 4), gen_v(0, 5),
                gen_q(0, 3), gen_v(0, 6), gen_v(0, 7),
                gen_q(0, 4), gen_v(1, 0), gen_v(1, 1),
                gen_q(0, 5), gen_v(1, 2), gen_v(1, 3),
                gen_q(1, 0), gen_v(1, 4), gen_v(1, 5),
                gen_q(1, 1), gen_v(1, 6), gen_v(1, 7),
                gen_q(1, 2), gen_q(1, 3), gen_q(1, 4), gen_q(1, 5),
            ])
            run_all(gen_q(0, 0, prio=0))
            run_all(gen_v(0, 0, prio=0))
            run_all(gen_v(0, 1, prio=0))
            blocks = [(hp, 0) for hp in range(HP)] + [(hp, 1) for hp in range(HP)]
            eb_tiles = deque([emit_eb(blocks[0][0])])
            pending_norm = deque()
            for ib, (hp, b) in enumerate(blocks):
                if ib + 1 < len(blocks):
                    eb_tiles.append(emit_eb(blocks[ib + 1][0]))
                pending_norm.append(
                    emit_attn(hp, b, eb_tiles.popleft(), filler, pending_norm))
                if b == 0 and hp == HP - 1:
                    # b=0 attention complete: its output projections become
                    # filler for the b=1 sweep
                    for tt in range(NT):
                        filler.append(gen_o(tt))
            while pending_norm:
                pending_norm.popleft()()
            while filler:
                pump(filler)
            for tt in range(NT, T // 128):
                run_all(gen_o(tt))

    nc.finalize()
    return nc


def _get_program():
    global _program
    if _program is None:
        _program = _build_program()
    return _program


def kernel(x, w_qv, ext_k, ext_bias, w_out, b_out):
    from concourse.bass_utils import run_bass_kernel_spmd

    nc = _get_program()
    bf = ml_dtypes.bfloat16

    x = np.asarray(x, dtype=np.float32)
    w_qv = np.asarray(w_qv, dtype=np.float32)
    ext_k = np.asarray(ext_k, dtype=np.float32)
    ext_bias = np.asarray(ext_bias, dtype=np.float32)
    w_out = np.asarray(w_out, dtype=np.float32)
    b_out = np.asarray(b_out, dtype=np.float32)

    w_q = np.ascontiguousarray(w_qv[:, :DIM] * SCALE).astype(bf)
    w_v = np.ascontiguousarray(w_qv[:, DIM:]).astype(bf)
    # kT packed head pairs: [6, 128, N]; rows 0:64 head 2hp, 64:128 head 2hp+1
    k0 = ext_k[0]                                    # [12, N, 64]
    kT = np.transpose(k0, (0, 2, 1)).reshape(HP, 128, N)
    kT = np.ascontiguousarray(kT).astype(bf)
    # exp(bias^T * scale): [12, m, n]
    eb = np.exp(np.transpose(ext_bias[0] * SCALE, (0, 2, 1)))
    eb = np.ascontiguousarray(eb).astype(bf)
    wo = np.ascontiguousarray(w_out).astype(bf)

    in_maps = []
    for c in range(CORES):
        xc = x[c * BPC:(c + 1) * BPC]                # [BPC, N, DIM]
        xT = np.ascontiguousarray(np.transpose(xc, (0, 2, 1))).astype(bf)
        in_maps.append({"xT": xT, "wq": w_q, "wv": w_v, "kT": kT,
                        "eb": eb, "wo": wo})

    res = run_bass_kernel_spmd(nc, in_maps, core_ids=list(range(CORES)))
    out = np.concatenate([res.results[c]["out"] for c in range(CORES)], axis=0)
    out = out.reshape(B, N, DIM) + b_out
    return out.astype(np.float32)


# revision 52
# speedup vs baseline: 1.0005x; 1.0005x over previous
"""Trainium2 Bass kernel for nn_Attention_66640712565145.

Attention with external K and full [h,n,n] bias:
  qv = x @ w_qv ; q,v = split(qv)
  dots = (q @ ext_k^T + ext_bias) * scale ; attn = softmax(dots)
  out = (attn @ v) @ w_out + b_out

Sharding: data-parallel over batch across 8 cores (2 batches/core), no
collectives.

Design notes (cost-model-driven; 434us -> 303us vs the v1 baseline):
- The bias add is removed from the PE entirely: the host ships
  exp(bias*scale) in bf16 and the unnormalized attention weights are
  formed as exp(S) * ebias with the multiply on DVE (bf16 2x mode).
  This removes one full S-sized pass through the PE (~82us/core).
- All matmul operands are bf16 (same PE cost as fp32r at >=256 free
  columns, half the DMA/SBUF), q^T stays resident in SBUF (no DRAM
  roundtrip), and PSUM->SBUF evictions are split between Act and DVE.
- One PSUM pool layout serves all phases: ps x3 [128,1024] + po
  [65,1024] = 8 banks. h2 is processed sequentially so only one PV
  accumulator is live, freeing a third ps slot that lets QK run ahead
  of the Act-paced exp pipeline and lets projection work interleave.
- PSUM slots are granted in *emission* order, so projection psum groups
  are emitted as generators yielding every 3 matmuls and pumped from
  inside the attention mt loop ("filler"), interleaving the b=1 input
  projections and b=0 output projections into attention-phase PE gaps.
- Row sums ride along the PV matmul via a ones column in the V tile; po
  is released early by unfused drain ops (reciprocal + bf16 copy);
  normalization (broadcast matmul + in-place multiply on a_sb) runs off
  the critical path.
- ebias streams per block as [128,1024] tiles on the SP/HWDGE queue;
  each block's normalization chain is emitted one block late so its
  PSUM slot request cannot throttle the next block's QK pipeline.
- Engine busy per core (cost model): PE 265us, Act 226us, DVE 203us,
  DMA 180us; wall 303us (PE ~90% occupied over its span).
"""
import numpy as np
import ml_dtypes

HEADS = 12
DIM_HEAD = 64
SCALE = DIM_HEAD ** -0.5
B, N, DIM = 16, 1024, 768
CORES = 8
BPC = B // CORES          # batches per core
T = BPC * N               # tokens per core
NT = N // 128             # 128-row tiles per sequence
VW = DIM_HEAD + 1         # v block width incl. ones column
ET = DIM // 128           # embedding chunks (6)
HP = HEADS // 2           # head pairs (6)

_program = None


def _build_program():
    import concourse.bacc as bacc
    import concourse.mybir as mybir
    from concourse.tile import TileContext

    dt = mybir.dt
    f32, f32r, bf16 = dt.float32, dt.float32r, dt.bfloat16
    EXP = mybir.ActivationFunctionType.Exp
    CPY = mybir.ActivationFunctionType.Copy

    nc = bacc.Bacc(None, target_bir_lowering=False)

    a_xT = nc.declare_dram_parameter("xT", [BPC, DIM, N], bf16, isOutput=False)
    a_wq = nc.declare_dram_parameter("wq", [DIM, DIM], bf16, isOutput=False)
    a_wv = nc.declare_dram_parameter("wv", [DIM, DIM], bf16, isOutput=False)
    a_kT = nc.declare_dram_parameter("kT", [HP, 128, N], bf16, isOutput=False)
    a_eb = nc.declare_dram_parameter("eb", [HEADS, N, N], bf16, isOutput=False)
    a_wo = nc.declare_dram_parameter("wo", [DIM, DIM], bf16, isOutput=False)
    a_out = nc.declare_dram_parameter("out", [T, DIM], f32, isOutput=True)

    with TileContext(nc) as tc:
        with tc.tile_pool(name="persist", bufs=1) as pers, \
             tc.tile_pool(name="wq_p", bufs=1) as wqpool, \
             tc.tile_pool(name="x_p", bufs=2) as xpool, \
             tc.tile_pool(name="eb_p", bufs=24) as ebpool, \
             tc.tile_pool(name="pt_p", bufs=6) as ptpool, \
             tc.tile_pool(name="qt_p", bufs=6) as qtpool, \
             tc.tile_pool(name="so_p", bufs=3) as sopool, \
             tc.tile_pool(name="psA", bufs=3, space="PSUM") as psA, \
             tc.tile_pool(name="psO", bufs=1, space="PSUM") as psO:

            # ---------- persistent SBUF ----------
            v_sb = pers.tile([128, BPC * NT * HEADS * VW], bf16, name="v_sb")
            with nc.allow_low_precision("ones col"):
                nc.vector.memset(
                    v_sb.rearrange("p (g w) -> p g w", w=VW)[:, :, DIM_HEAD:VW]
                        .rearrange("p g w -> p (g w)"), 1.0)
            a_sb = pers.tile([128, ET * T], bf16, name="a_sb")
            kT_sb = pers.tile([128, HP * N], bf16, name="kT_sb")
            l_sb = pers.tile([128, N], f32r, name="l_sb")
            nc.vector.memset(l_sb.bitcast(f32), 1.0)
            ind_f = pers.tile([128, 128], f32, name="ind_f")
            ind_r = []
            for bb in range(BPC):
                nc.vector.memset(ind_f, 0.0)
                nc.vector.memset(ind_f[(bb * 2) * 32:(bb * 2) * 32 + 1, 0:64], 1.0)
                nc.vector.memset(
                    ind_f[(bb * 2 + 1) * 32:(bb * 2 + 1) * 32 + 1, 64:128], 1.0)
                t5 = pers.tile([128, 128], f32r, name=f"ind_r{bb}")
                with nc.allow_low_precision("exact const"):
                    nc.vector.tensor_copy(t5, ind_f)
                ind_r.append(t5)

            # ---------- input DMAs (order matters: x(b0) + wq first) ----------
            xT_t = {}
            wq_t, wv_t, wo_t = [], [], []

            def dma_x(b, ct):
                t3 = xpool.tile([128, N], bf16, name=f"xT{ct}")
                nc.sync.dma_start(out=t3, in_=a_xT[b, ct * 128:(ct + 1) * 128, :])
                xT_t[(b, ct)] = t3

            dma_x(0, 0)
            for ct in range(ET):
                t1 = wqpool.tile([128, DIM], bf16, name=f"wq{ct}")
                nc.sync.dma_start(out=t1, in_=a_wq[ct * 128:(ct + 1) * 128, :])
                wq_t.append(t1)
            for ct in range(1, ET):
                dma_x(0, ct)
            nc.sync.dma_start(out=kT_sb[:, 0:N], in_=a_kT[0, :, :])
            for ct in range(ET):
                t2 = wqpool.tile([128, DIM], bf16, name=f"wv{ct}")
                nc.sync.dma_start(out=t2, in_=a_wv[ct * 128:(ct + 1) * 128, :])
                wv_t.append(t2)
            for hp in range(1, HP):
                nc.sync.dma_start(out=kT_sb[:, hp * N:(hp + 1) * N],
                                  in_=a_kT[hp, :, :])
            for ct in range(ET):
                dma_x(1, ct)
            for et in range(ET):
                t4 = wqpool.tile([128, DIM], bf16, name=f"wo{et}")
                nc.sync.dma_start(out=t4, in_=a_wo[et * 128:(et + 1) * 128, :])
                wo_t.append(t4)

            # ---------- emission helpers ----------
            # Projection groups are generators yielding every few matmuls so
            # their PE work can be sprinkled between attention tiles (PE pulls
            # ready work in priority=emission order, so a contiguous 12-matmul
            # group would stall the next QK by ~2.6us and starve Act).
            qt = {}

            def gen_q(b, et, prio=-150):
                pq = psA.tile([128, N], f32, name="ps")
                qt[(b, et)] = qtpool.tile([128, N], bf16, name="qt")
                k = 0
                for ct in range(ET):
                    for nch in range(2):
                        nsl = slice(nch * 512, (nch + 1) * 512)
                        with tc.high_priority(offset=prio):
                            nc.tensor.matmul(
                                pq[:, nsl],
                                wq_t[ct][:, et * 128:(et + 1) * 128],
                                xT_t[(b, ct)][:, nsl],
                                start=(ct == 0), stop=(ct == ET - 1))
                        k += 1
                        if k % 3 == 0:
                            yield
                with tc.high_priority(offset=prio), \
                     nc.allow_low_precision("bf16 q"):
                    nc.scalar.activation(qt[(b, et)], pq, CPY)

            def gen_v(b, tt, prio=-150):
                pv = psA.tile([128, N], f32, name="ps")
                k = 0
                for ct in range(ET):
                    for osl in (slice(0, 512), slice(512, DIM)):
                        with tc.high_priority(offset=prio):
                            nc.tensor.matmul(
                                pv[:, osl],
                                xT_t[(b, ct)][:, tt * 128:(tt + 1) * 128],
                                wv_t[ct][:, osl],
                                start=(ct == 0), stop=(ct == ET - 1))
                        k += 1
                        if k % 3 == 0:
                            yield
                base = (b * NT + tt) * HEADS * VW
                dst3 = v_sb[:, base:base + HEADS * VW] \
                    .rearrange("p (h w) -> p h w", w=VW)
                with nc.allow_low_precision("bf16 v"):
                    nc.vector.tensor_copy(
                        dst3[:, :, 0:DIM_HEAD],
                        pv[:, 0:DIM].rearrange("p (h w) -> p h w", w=DIM_HEAD))

            def gen_o(tt, prio=-150):
                pp = psA.tile([128, N], f32, name="ps")
                k = 0
                for et in range(ET):
                    for osl in (slice(0, 512), slice(512, DIM)):
                        with tc.high_priority(offset=prio):
                            nc.tensor.matmul(
                                pp[:, osl],
                                a_sb[:, et * T + tt * 128:et * T + (tt + 1) * 128],
                                wo_t[et][:, osl],
                                start=(et == 0), stop=(et == ET - 1))
                        k += 1
                        if k % 3 == 0:
                            yield
                so = sopool.tile([128, DIM], f32, name="so")
                nc.scalar.activation(so, pp[:, 0:DIM], CPY)
                nc.sync.dma_start(out=a_out[tt * 128:(tt + 1) * 128, :], in_=so)

            _DONE = object()

            def pump(filler):
                while filler:
                    if next(filler[0], _DONE) is _DONE:
                        filler.popleft()
                    else:
                        return

            def run_all(gen):
                for _ in gen:
                    pass

            def emit_eb(hp):
                tiles = {}
                for h2 in range(2):
                    h = 2 * hp + h2
                    for mt in range(NT):
                        te = ebpool.tile([128, N], bf16, name="ebt")
                        nc.sync.dma_start(
                            out=te,
                            in_=a_eb[h, mt * 128:(mt + 1) * 128, :])
                        tiles[(h2, mt)] = te
                return tiles

            def emit_attn(hp, b, ebt, filler, pending_norm):
                tq = qt[(b, hp)]
                for h2 in range(2):
                    h = 2 * hp + h2
                    r0 = h2 * 64
                    po = psO.tile([VW, N], f32, name="po")
                    for mt in range(NT):
                        pump(filler)
                        if h2 == 0 and mt == 5 and pending_norm:
                            pending_norm.popleft()()
                        pss = psA.tile([128, N], f32, name="ps")
                        for nch in range(2):
                            nsl = slice(nch * 512, (nch + 1) * 512)
                            nc.tensor.matmul(
                                pss[:, nsl],
                                kT_sb[r0:r0 + 64,
                                      hp * N + mt * 128:hp * N + (mt + 1) * 128],
                                tq[r0:r0 + 64, nsl],
                                start=True, stop=True,
                                tile_position=(r0, 0))
                        pt = ptpool.tile([128, N], bf16, name="pt")
                        with nc.allow_low_precision("bf16 exp"):
                            nc.scalar.activation(pt, pss, EXP)
                        ebs = ebt[(h2, mt)]
                        with nc.allow_low_precision("attn*ebias"):
                            nc.vector.tensor_mul(pt, pt, ebs)
                        vbase = (b * NT + mt) * HEADS * VW + h * VW
                        for nch in range(2):
                            nsl = slice(nch * 512, (nch + 1) * 512)
                            nc.tensor.matmul(
                                po[:, nsl],
                                v_sb[:, vbase:vbase + VW],
                                pt[:, nsl],
                                start=(mt == 0), stop=(mt == NT - 1))
                    # drain: reciprocal of the ones-row + unnormalized copy
                    # (the two last readers of po — releases it for h2+1).
                    # High priority so they jump ahead of queued DVE multiplies.
                    with tc.high_priority(offset=200):
                        with nc.allow_low_precision("softmax recip"):
                            nc.vector.reciprocal(
                                l_sb[(b * 2 + h2) * 32:(b * 2 + h2) * 32 + 1, :],
                                po[DIM_HEAD:VW, :])
                        with nc.allow_low_precision("bf16 attn out"):
                            nc.vector.tensor_copy(
                                a_sb[h2 * 64:(h2 + 1) * 64,
                                     hp * T + b * N:hp * T + (b + 1) * N],
                                po[0:DIM_HEAD, :])
                def norm(hp=hp, b=b):
                    pb = psA.tile([128, N], f32, name="ps")
                    for nch in range(2):
                        nsl = slice(nch * 512, (nch + 1) * 512)
                        nc.tensor.matmul(
                            pb[:, nsl], ind_r[b], l_sb[:, nsl],
                            start=True, stop=True)
                    asl = a_sb[:, hp * T + b * N:hp * T + (b + 1) * N]
                    with nc.allow_low_precision("normalize"):
                        nc.vector.tensor_mul(asl, asl, pb)
                return norm

            # ---------- phase emission ----------
            # Block order: all of b=0's head-pairs, then all of b=1's.
            # b=1 projections and b=0 output-projections run as woven
            # "filler" psum-groups popped inside the attention mt loops
            # (PSUM slots are granted in emission order, so this is the
            # only way projection work overlaps the Act-paced pipeline).
            from collections import deque
            filler = deque([
                gen_q(0, 1), gen_v(0, 2), gen_v(0, 3),
                gen_q(0, 2), gen_v(0, 4), gen_v(0, 5),
                gen_q(0, 3), gen_v(0, 6), gen_v(0, 7),
                gen_q(0, 4), gen_v(1, 0), gen_v(1, 1),
                gen_q(0, 5), gen_v(1, 2), gen_v(1, 3),
                gen_q(1, 0), gen_v(1, 4), gen_v(1, 5),
                gen_q(1, 1), gen_v(1, 6), gen_v(1, 7),
                gen_q(1, 2), gen_q(1, 3), gen_q(1, 4), gen_q(1, 5),
            ])
            run_all(gen_q(0, 0, prio=0))
            run_all(gen_v(0, 0, prio=0))
            run_all(gen_v(0, 1, prio=0))
            blocks = [(hp, 0) for hp in range(HP)] + [(hp, 1) for hp in range(HP)]
            eb_tiles = deque([emit_eb(blocks[0][0])])
            pending_norm = deque()
            for ib, (hp, b) in enumerate(blocks):
                if ib + 1 < len(blocks):
                    eb_tiles.append(emit_eb(blocks[ib + 1][0]))
                pending_norm.append(
                    emit_attn(hp, b, eb_tiles.popleft(), filler, pending_norm))
                if b == 0 and hp == HP - 1:
                    # b=0 attention complete: its output projections become
                    # filler for the b=1 sweep
                    for tt in range(NT):
                        filler.append(gen_o(tt))
            while pending_norm:
                pending_norm.popleft()()
            while filler:
                pump(filler)
            for tt in range(NT, T // 128):
                run_all(gen_o(tt))

    nc.finalize()
    return nc


def _get_program():
    global _program
    if _program is None:
        _program = _build_program()
    return _program


def kernel(x, w_qv, ext_k, ext_bias, w_out, b_out):
    from concourse.bass_utils import run_bass_kernel_spmd

    nc = _get_program()
    bf = ml_dtypes.bfloat16

    x = np.asarray(x, dtype=np.float32)
    w_qv = np.asarray(w_qv, dtype=np.float32)
    ext_k = np.asarray(ext_k, dtype=np.float32)
    ext_bias = np.asarray(ext_bias, dtype=np.float32)
    w_out = np.asarray(w_out, dtype=np.float32)
    b_out = np.asarray(b_out, dtype=np.float32)

    w_q = np.ascontiguousarray(w_qv[:, :DIM] * SCALE).astype(bf)
    w_v = np.ascontiguousarray(w_qv[:, DIM:]).astype(bf)
    # kT packed head pairs: [6, 128, N]; rows 0:64 head 2hp, 64:128 head 2hp+1
    k0 = ext_k[0]                                    # [12, N, 64]
    kT = np.transpose(k0, (0, 2, 1)).reshape(HP, 128, N)
    kT = np.ascontiguousarray(kT).astype(bf)
    # exp(bias^T * scale): [12, m, n]
    eb = np.exp(np.transpose(ext_bias[0] * SCALE, (0, 2, 1)))
    eb = np.ascontiguousarray(eb).astype(bf)
    wo = np.ascontiguousarray(w_out).astype(bf)

    in_maps = []
    for c in range(CORES):
        xc = x[c * BPC:(c + 1) * BPC]                # [BPC, N, DIM]
        xT = np.ascontiguousarray(np.transpose(xc, (0, 2, 1))).astype(bf)
        in_maps.append({"xT": xT, "wq": w_q, "wv": w_v, "kT": kT,
                        "eb": eb, "wo": wo})

    res = run_bass_kernel_spmd(nc, in_maps, core_ids=list(range(CORES)))
    out = np.concatenate([res.results[c]["out"] for c in range(CORES)], axis=0)
    out = out.reshape(B, N, DIM) + b_out
    return out.astype(np.float32)
